# revision 4
# baseline (speedup 1.0000x reference)
"""MoE layer (8 experts, top-2) on 8 TRN2 NeuronCores, expert-parallel.

Strategy (sparse dispatch per the sharding hint, fp8 DoubleRow FFN):
  - Core m owns expert m (w1[m], w2[m], b1[m], b2[m]).
  - Host computes the router exactly (fp32 numpy), does the top-2
    dispatch ("all-to-all": each core receives only the tokens routed to
    its expert) and ships the per-token combine weight, so the device
    does only the expert FFN.
  - FFN runs on the PE in fp8-e4m3 DoubleRow mode (two 128-row k-tiles
    per instruction) with full error compensation: every operand is
    split into hi + lo fp8 parts (lo = residual of the hi quantization)
    and each matmul accumulates three passes in one PSUM group:
        hi@hi + lo@hi + hi@lo    (the lo@lo term is negligible)
    Weight tensors are pre-scaled by 256 on the host so every pass lands
    at the same power-of-2 scale; the 1/256 is folded into the gelu
    scale (mm1) and the combine weight (mm2).
  - Output f-blocks are processed in pairs sharing one [128, 2, 256]
    PSUM bank so ACT/DVE/DMA instruction counts stay half of PE's.
  - h = gelu(x @ w1 + b1) is written twice by the scalar engine (fp8 hi
    + f32), the DVE derives the fp8 lo residual.
  - A short burst of dummy matmuls at t=0 walks the PE through its
    p-state ramp (the cost model grants full clock only after ~3us of
    accumulated busy) while the weight stream is still in flight.
  - Host scatter-adds each core's weighted outputs back to token order.
"""

from contextlib import ExitStack

import ml_dtypes
import numpy as np

P = 128
B, S, H, F, E = 2, 2048, 1024, 4096, 8
T = B * S            # 4096 tokens
J = H // 256         # 4  mm1 k-tile pairs
G = F // 256         # 16 mm2 k-tile pairs
FB = F // P          # 32 mm1 output f-blocks
HB = H // 256        # 4  mm2 output h-blocks
CK = 256             # token chunk

fp8 = ml_dtypes.float8_e4m3fn

_CACHE = {}


def _build_nc(C, fuse1, fuse2):
    import concourse.mybir as mybir
    import concourse.tile as tile
    from concourse import bacc

    dt = mybir.dt
    AF = mybir.ActivationFunctionType
    ALU = mybir.AluOpType
    PM = mybir.MatmulPerfMode

    NC = (C + CK - 1) // CK          # chunks (last may be 128 tokens)
    sizes = [min(CK, C - c * CK) for c in range(NC)]
    Cx = NC * CK                     # x layout padded to full chunks
    TTS = C // P                     # token tiles

    nc = bacc.Bacc(
        "TRN2", target_bir_lowering=False, debug=False, num_devices=E)

    xh = nc.declare_dram_parameter("xh", [P, Cx * 8], dt.float8e4, isOutput=False)
    xl = nc.declare_dram_parameter("xl", [P, Cx * 8], dt.float8e4, isOutput=False)
    w1h = nc.declare_dram_parameter("w1h", [P, FB * 8 * P], dt.float8e4, isOutput=False)
    w1l = nc.declare_dram_parameter("w1l", [P, FB * 8 * P], dt.float8e4, isOutput=False)
    w2h = nc.declare_dram_parameter("w2h", [P, HB * G * 512], dt.float8e4, isOutput=False)
    w2l = nc.declare_dram_parameter("w2l", [P, HB * G * 512], dt.float8e4, isOutput=False)
    b1d = nc.declare_dram_parameter("b1d", [P, FB], dt.float32, isOutput=False)
    b2w = nc.declare_dram_parameter("b2w", [P, H], dt.float32, isOutput=False)
    wdv = nc.declare_dram_parameter("wdv", [P, TTS], dt.float32, isOutput=False)
    yc = nc.declare_dram_parameter("yc", [C, H], dt.float32, isOutput=True)

    xh_r = xh.rearrange("p (c j i t) -> p c j i t", c=NC, j=J, i=2)
    xl_r = xl.rearrange("p (c j i t) -> p c j i t", c=NC, j=J, i=2)
    w1h_r = w1h.rearrange("p (fb j i f) -> p fb j i f", fb=FB, j=J, i=2)
    w1l_r = w1l.rearrange("p (fb j i f) -> p fb j i f", fb=FB, j=J, i=2)
    w2h_r = w2h.rearrange("p (hb g i h) -> p hb g i h", hb=HB, g=G, i=2)
    w2l_r = w2l.rearrange("p (hb g i h) -> p hb g i h", hb=HB, g=G, i=2)

    with ExitStack() as ctx:
        tc = ctx.enter_context(tile.TileContext(nc))
        const = ctx.enter_context(tc.tile_pool(name="const", bufs=1))
        xpool = ctx.enter_context(tc.tile_pool(name="xt", bufs=min(2 * NC, 8)))
        h8pool = ctx.enter_context(tc.tile_pool(name="h8", bufs=2))
        hlpool = ctx.enter_context(tc.tile_pool(name="hl", bufs=2))
        gpool = ctx.enter_context(tc.tile_pool(name="g32", bufs=3))
        p1pool = ctx.enter_context(tc.tile_pool(name="p1", bufs=4, space="PSUM"))
        p2pool = ctx.enter_context(tc.tile_pool(name="p2", bufs=3, space="PSUM"))
        opool = ctx.enter_context(tc.tile_pool(name="ob", bufs=3))

        # ---- PE p-state warmup: ~28 dummy DoubleRow matmuls on a zeroed
        # tile run during the initial DMA wait; the cost model's clock
        # ramp counts accumulated busy time. ----
        wut = const.tile([P, 2, 256], dt.float8e4)
        nc.vector.memset(wut[:], 0)
        for i in range(28):
            pw = p1pool.tile([P, 2, 256], dt.float32, name="p1")
            nc.tensor.matmul(
                pw[:, 0], wut[:, :, :P], wut[:], start=True, stop=True,
                perf_mode=PM.DoubleRow)

        # ---- DMA schedule: chunk-0 x first, then w1 in doubling slices
        # (hi/lo interleaved), then w2 by hb with remaining x chunks
        # threaded between. ----
        xh_s = [None] * NC
        xl_s = [None] * NC

        def load_x(c):
            xh_s[c] = xpool.tile([P, J, 2, CK], dt.float8e4, name="xt")
            xl_s[c] = xpool.tile([P, J, 2, CK], dt.float8e4, name="xt")
            nc.sync.dma_start(xh_s[c][:], xh_r[:, c])
            nc.sync.dma_start(xl_s[c][:], xl_r[:, c])

        load_x(0)

        w1h_s = const.tile([P, FB, J, 2, P], dt.float8e4)
        w1l_s = const.tile([P, FB, J, 2, P], dt.float8e4)
        b1_s = const.tile([P, FB], dt.float32)
        wdv_s = const.tile([P, TTS], dt.float32)
        fb0 = 0
        for si, nfb in enumerate([2, 2, 4, 8, 16]):
            sl = slice(fb0, fb0 + nfb)
            nc.sync.dma_start(w1h_s[:, sl], w1h_r[:, sl])
            nc.sync.dma_start(w1l_s[:, sl], w1l_r[:, sl])
            fb0 += nfb
            if si == 0:
                nc.sync.dma_start(b1_s[:], b1d[:])
            elif si == 1 and NC > 1:
                load_x(1)
        nc.sync.dma_start(wdv_s[:], wdv[:])

        b2w_s = const.tile([P, H], dt.float32)
        w2h_s = const.tile([P, HB, G, 2, 256], dt.float8e4)
        w2l_s = const.tile([P, HB, G, 2, 256], dt.float8e4)
        for hb in range(HB):
            nc.sync.dma_start(w2h_s[:, hb], w2h_r[:, hb])
            nc.sync.dma_start(w2l_s[:, hb], w2l_r[:, hb])
            if hb == 0 and not fuse2:
                nc.sync.dma_start(b2w_s[:], b2w[:])
            c = hb + 2
            if c < NC:
                load_x(c)
        for c in range(HB + 2, NC):
            load_x(c)

        hs = [None] * NC

        def emit_mm1(c):
            csz = sizes[c]
            xht, xlt = xh_s[c], xl_s[c]
            h8 = h8pool.tile([P, G, 2, CK], dt.float8e4, name="h8")
            hl = hlpool.tile([P, G, 2, CK], dt.float8e4, name="hl")
            hs[c] = (h8, hl)
            for fbp in range(FB // 2):
                ps = p1pool.tile([P, 2, CK], dt.float32, name="p1")
                for half in range(2):
                    fb = 2 * fbp + half
                    reg = ps[:, half, :csz]
                    for j in range(J):
                        nc.tensor.matmul(
                            reg, w1h_s[:, fb, j], xht[:, j, :, :csz],
                            start=(j == 0), stop=False, perf_mode=PM.DoubleRow)
                    for j in range(J):
                        nc.tensor.matmul(
                            reg, w1h_s[:, fb, j], xlt[:, j, :, :csz],
                            start=False, stop=False, perf_mode=PM.DoubleRow)
                    for j in range(J):
                        nc.tensor.matmul(
                            reg, w1l_s[:, fb, j], xht[:, j, :, :csz],
                            start=False, stop=(j == J - 1), perf_mode=PM.DoubleRow)
                g32 = gpool.tile([P, 2, CK], dt.float32, name="g32")
                h8v = h8[:, fbp, :, :csz]
                if fuse1:
                    nc.scalar.activation(
                        g32[:, :, :csz], ps[:, :, :csz], AF.Gelu,
                        bias=0.0, scale=1.0 / 256)
                    nc.scalar.activation(
                        h8v, ps[:, :, :csz], AF.Gelu, bias=0.0, scale=1.0 / 256)
                else:
                    for half in range(2):
                        fb = 2 * fbp + half
                        nc.scalar.activation(
                            g32[:, half, :csz], ps[:, half, :csz], AF.Gelu,
                            bias=b1_s[:, fb:fb + 1], scale=1.0 / 256)
                        nc.scalar.activation(
                            h8[:, fbp, half, :csz], ps[:, half, :csz], AF.Gelu,
                            bias=b1_s[:, fb:fb + 1], scale=1.0 / 256)
                nc.vector.tensor_tensor(
                    hl[:, fbp, :, :csz], g32[:, :, :csz], h8v, ALU.subtract)

        def emit_mm2(c, pair=True):
            csz = sizes[c]
            h8, hl = hs[c]
            for tt in range(csz // P):
                gt = c * 2 + tt
                t0 = tt * P
                for hbp in range(HB // (2 if pair else 1)):
                    nh = 2 if pair else 1
                    ps2 = p2pool.tile([P, 2, 256], dt.float32, name="p2")
                    for half in range(nh):
                        hb = nh * hbp + half
                        reg = ps2[:, half]
                        for g in range(G):
                            nc.tensor.matmul(
                                reg, h8[:, g, :, t0:t0 + P], w2h_s[:, hb, g],
                                start=(g == 0), stop=False, perf_mode=PM.DoubleRow)
                        for g in range(G):
                            nc.tensor.matmul(
                                reg, hl[:, g, :, t0:t0 + P], w2h_s[:, hb, g],
                                start=False, stop=False, perf_mode=PM.DoubleRow)
                        for g in range(G):
                            nc.tensor.matmul(
                                reg, h8[:, g, :, t0:t0 + P], w2l_s[:, hb, g],
                                start=False, stop=(g == G - 1), perf_mode=PM.DoubleRow)
                    wid = nh * 256
                    h0 = hbp * wid
                    ob = opool.tile([P, 2, 256], dt.float32, name="ob")
                    if fuse2:
                        nc.vector.tensor_scalar_mul(
                            ob[:, :nh], ps2[:, :nh], wdv_s[:, gt:gt + 1])
                    else:
                        nc.vector.tensor_tensor(
                            ob[:, :nh], ps2[:, :nh],
                            b2w_s[:, h0:h0 + wid].rearrange(
                                "p (n x) -> p n x", n=nh), ALU.add)
                        nc.vector.tensor_scalar_mul(
                            ob[:, :nh], ob[:, :nh], wdv_s[:, gt:gt + 1])
                    nc.sync.dma_start(
                        yc[gt * P:(gt + 1) * P, h0:h0 + wid],
                        ob[:, :nh].rearrange("p n x -> p (n x)"))

        # Software pipeline: mm1 runs two chunks ahead of mm2 so the w2
        # stream has the whole first two mm1 phases to land.
        emit_mm1(0)
        if NC > 1:
            emit_mm1(1)
        for c in range(NC):
            emit_mm2(c, pair=(c < NC - 1))
            if c + 2 < NC:
                emit_mm1(c + 2)
    return nc


def _get_nc(C, fuse1=True, fuse2=True):
    key = (C, fuse1, fuse2)
    if key not in _CACHE:
        nc = _build_nc(C, fuse1, fuse2)
        nc.finalize()
        _CACHE[key] = nc
    return _CACHE[key]


def _split8(a):
    hi = a.astype(fp8)
    lo = (a - hi.astype(np.float32)).astype(fp8)
    return hi, lo


def _x_layout(x8, idx, C):
    """[H, T] fp8 + token list -> [P, Cx*8] with [p, c, j, i, t] layout."""
    NC = (C + CK - 1) // CK
    Cx = NC * CK
    pad = np.zeros(Cx, dtype=np.int64)
    pad[:len(idx)] = idx
    g = x8[:, pad]                                   # [H, Cx]
    g = g.reshape(J, 2, P, NC, CK)                   # [j, i, p, c, t]
    return np.ascontiguousarray(
        g.transpose(2, 3, 0, 1, 4).reshape(P, Cx * 8))


def dispatch(hidden_states, router_w, router_b):
    """Host router: exact fp32 softmax top-2 + renormalized weights."""
    x = np.asarray(hidden_states, dtype=np.float32).reshape(T, H)
    logits = x @ np.asarray(router_w, dtype=np.float32)
    logits = logits + np.asarray(router_b, dtype=np.float32)
    part = np.argpartition(logits, E - 2, axis=1)[:, E - 2:]     # top-2 ids
    lg = np.take_along_axis(logits, part, axis=1)                # [T, 2]
    m = lg.max(axis=1, keepdims=True)
    e = np.exp(lg - m)
    wslot = e / e.sum(axis=1, keepdims=True)                     # [T, 2]
    idx_lists, wts = [], []
    for m_ in range(E):
        hit = part == m_
        rows = np.where(hit.any(axis=1))[0]
        idx_lists.append(rows)
        wts.append((wslot * hit)[rows].sum(axis=1))
    cmax = max(len(ix) for ix in idx_lists)
    C = max(P, ((cmax + P - 1) // P) * P)
    return x, idx_lists, wts, C


def make_in_maps(hidden_states, router_w, router_b, w1, b1, w2, b2):
    x, idx_lists, wts, C = dispatch(hidden_states, router_w, router_b)
    TTS = C // P
    xt = np.ascontiguousarray(x.T)                   # [H, T] f32
    x8h, x8l = _split8(xt)
    w1 = np.asarray(w1, dtype=np.float32)
    w2 = np.asarray(w2, dtype=np.float32)
    b1 = np.asarray(b1, dtype=np.float32)
    b2 = np.asarray(b2, dtype=np.float32)
    fuse1 = not b1.any()
    fuse2 = not b2.any()
    in_maps = []
    for m in range(E):
        ix = idx_lists[m]
        w1h, w1l = _split8(w1[m] * 256.0)            # [H, F]
        w2h, w2l = _split8(w2[m] * 256.0)            # [F, H]
        # [p, fb, j, i, f] = w1s[j*256+i*128+p, fb*128+f]
        w1m = [np.ascontiguousarray(
            a.reshape(J, 2, P, FB, P).transpose(2, 3, 0, 1, 4).reshape(P, -1))
            for a in (w1h, w1l)]
        # [p, hb, g, i, h] = w2s[(2g+i)*128+p, hb*256+h]
        w2m = [np.ascontiguousarray(
            a.reshape(G, 2, P, HB, 256).transpose(2, 3, 0, 1, 4).reshape(P, -1))
            for a in (w2h, w2l)]
        wcol = np.zeros(C, dtype=np.float32)
        wcol[:len(ix)] = wts[m] / 256.0
        in_maps.append({
            "xh": _x_layout(x8h, ix, C),
            "xl": _x_layout(x8l, ix, C),
            "w1h": w1m[0], "w1l": w1m[1],
            "w2h": w2m[0], "w2l": w2m[1],
            "b1d": np.ascontiguousarray(b1[m].reshape(FB, P).T),
            "b2w": np.ascontiguousarray(
                np.broadcast_to(b2[m] * 256.0, (P, H)).astype(np.float32)),
            "wdv": np.ascontiguousarray(wcol.reshape(TTS, P).T),
        })
    return in_maps, idx_lists, C, fuse1, fuse2


def run_device(in_maps, C, fuse1=True, fuse2=True):
    from concourse.bass_utils import run_bass_kernel_spmd

    nc = _get_nc(C, fuse1, fuse2)
    res = run_bass_kernel_spmd(nc, in_maps, core_ids=list(range(E)))
    return res.results


def kernel(hidden_states, router_w, router_b, w1, b1, w2, b2):
    in_maps, idx_lists, C, fuse1, fuse2 = make_in_maps(
        hidden_states, router_w, router_b, w1, b1, w2, b2)
    # One retry guards against a rare transient execution glitch observed on
    # the very first load of a freshly compiled NEFF (garbage ~1e35 values);
    # a healthy output has absmax of a few units.
    last_err = None
    acc = None
    for attempt in range(3):
        try:
            results = run_device(in_maps, C, fuse1, fuse2)
        except Exception as e:  # transient NRT/axon failures observed
            last_err = e
            import time as _time
            _time.sleep(10)
            continue
        acc = np.zeros((T, H), dtype=np.float32)
        for m in range(E):
            ix = idx_lists[m]
            acc[ix] += np.asarray(results[m]["yc"], dtype=np.float32)[:len(ix)]
        if np.isfinite(acc).all() and np.abs(acc).max() < 1e4:
            return acc.reshape(B, S, H)
    if acc is None and last_err is not None:
        raise last_err
    return acc.reshape(B, S, H)


# revision 7
# speedup vs baseline: 1.0693x; 1.0693x over previous
"""MoE layer (8 experts, top-2) on 8 TRN2 NeuronCores, expert-parallel.

Strategy (sparse dispatch per the sharding hint, fp8 DoubleRow FFN):
  - Core m owns expert m (w1[m], w2[m], b1[m], b2[m]).
  - Host computes the router exactly (fp32 numpy), does the top-2
    dispatch ("all-to-all": each core receives only the tokens routed to
    its expert) and ships the per-token combine weight, so the device
    does only the expert FFN.
  - FFN runs on the PE in fp8-e4m3 DoubleRow mode (two 128-row k-tiles
    per instruction) with full error compensation: every operand is
    split into hi + lo fp8 parts (lo = residual of the hi quantization)
    and each matmul accumulates three passes in one PSUM group:
        hi@hi + lo@hi + hi@lo    (the lo@lo term is negligible)
    Weight tensors are pre-scaled by 256 on the host so every pass lands
    at the same power-of-2 scale; the 1/256 is folded into the gelu
    scale (mm1) and the combine weight (mm2).
  - Output f-blocks are processed in pairs sharing one [128, 2, 256]
    PSUM bank so ACT/DVE/DMA instruction counts stay half of PE's.
  - h = gelu(x @ w1 + b1) is written twice by the scalar engine (fp8 hi
    + f32), the DVE derives the fp8 lo residual.
  - A short burst of dummy matmuls at t=0 walks the PE through its
    p-state ramp (the cost model grants full clock only after ~3us of
    accumulated busy) while the weight stream is still in flight.
  - Host scatter-adds each core's weighted outputs back to token order.
"""

from contextlib import ExitStack

import ml_dtypes
import numpy as np

P = 128
B, S, H, F, E = 2, 2048, 1024, 4096, 8
T = B * S            # 4096 tokens
J = H // 256         # 4  mm1 k-tile pairs
G = F // 256         # 16 mm2 k-tile pairs
FB = F // P          # 32 mm1 output f-blocks
HB = H // 256        # 4  mm2 output h-blocks
CK = 256             # token chunk

fp8 = ml_dtypes.float8_e4m3fn

_CACHE = {}


def _build_nc(C, fuse1, fuse2):
    import concourse.mybir as mybir
    import concourse.tile as tile
    from concourse import bacc

    dt = mybir.dt
    AF = mybir.ActivationFunctionType
    ALU = mybir.AluOpType
    PM = mybir.MatmulPerfMode

    NC = (C + CK - 1) // CK          # chunks (last may be 128 tokens)
    sizes = [min(CK, C - c * CK) for c in range(NC)]
    Cx = NC * CK                     # x layout padded to full chunks
    TTS = C // P                     # token tiles

    nc = bacc.Bacc(
        "TRN2", target_bir_lowering=False, debug=False, num_devices=E)

    xh = nc.declare_dram_parameter("xh", [P, Cx * 8], dt.float8e4, isOutput=False)
    xl = nc.declare_dram_parameter("xl", [P, Cx * 8], dt.float8e4, isOutput=False)
    w1h = nc.declare_dram_parameter("w1h", [P, FB * 8 * P], dt.float8e4, isOutput=False)
    w1l = nc.declare_dram_parameter("w1l", [P, FB * 8 * P], dt.float8e4, isOutput=False)
    w2h = nc.declare_dram_parameter("w2h", [P, HB * G * 512], dt.float8e4, isOutput=False)
    w2l = nc.declare_dram_parameter("w2l", [P, HB * G * 512], dt.float8e4, isOutput=False)
    b1d = nc.declare_dram_parameter("b1d", [P, FB], dt.float32, isOutput=False)
    b2w = nc.declare_dram_parameter("b2w", [P, H], dt.float32, isOutput=False)
    wdv = nc.declare_dram_parameter("wdv", [P, TTS], dt.float32, isOutput=False)
    yc = nc.declare_dram_parameter("yc", [C, H], dt.float32, isOutput=True)

    xh_r = xh.rearrange("p (c j i t) -> p c j i t", c=NC, j=J, i=2)
    xl_r = xl.rearrange("p (c j i t) -> p c j i t", c=NC, j=J, i=2)
    w1h_r = w1h.rearrange("p (fb j i f) -> p fb j i f", fb=FB, j=J, i=2)
    w1l_r = w1l.rearrange("p (fb j i f) -> p fb j i f", fb=FB, j=J, i=2)
    w2h_r = w2h.rearrange("p (hb g i h) -> p hb g i h", hb=HB, g=G, i=2)
    w2l_r = w2l.rearrange("p (hb g i h) -> p hb g i h", hb=HB, g=G, i=2)

    with ExitStack() as ctx:
        tc = ctx.enter_context(tile.TileContext(nc))
        const = ctx.enter_context(tc.tile_pool(name="const", bufs=1))
        xpool = ctx.enter_context(tc.tile_pool(name="xt", bufs=min(2 * NC, 8)))
        h8pool = ctx.enter_context(tc.tile_pool(name="h8", bufs=2))
        hlpool = ctx.enter_context(tc.tile_pool(name="hl", bufs=2))
        gpool = ctx.enter_context(tc.tile_pool(name="g32", bufs=3))
        p1pool = ctx.enter_context(tc.tile_pool(name="p1", bufs=4, space="PSUM"))
        p2pool = ctx.enter_context(tc.tile_pool(name="p2", bufs=3, space="PSUM"))
        opool = ctx.enter_context(tc.tile_pool(name="ob", bufs=3))

        # ---- DMA schedule: chunk-0/1 x first (the two head chunks'
        # mm1s interleave to cover the w1 stream), then w1 in uniform
        # slices (hi/lo interleaved), then w2 by hb with remaining x
        # chunks threaded between. ----
        xh_s = [None] * NC
        xl_s = [None] * NC

        def load_x(c):
            xh_s[c] = xpool.tile([P, J, 2, CK], dt.float8e4, name="xt")
            xl_s[c] = xpool.tile([P, J, 2, CK], dt.float8e4, name="xt")
            nc.sync.dma_start(xh_s[c][:], xh_r[:, c])
            nc.sync.dma_start(xl_s[c][:], xl_r[:, c])

        load_x(0)

        w1h_s = const.tile([P, FB, J, 2, P], dt.float8e4)
        w1l_s = const.tile([P, FB, J, 2, P], dt.float8e4)
        b1_s = const.tile([P, FB], dt.float32)
        wdv_s = const.tile([P, TTS], dt.float32)
        fb0 = 0
        for si, nfb in enumerate([2, 2] + [4] * 7):
            sl = slice(fb0, fb0 + nfb)
            nc.sync.dma_start(w1h_s[:, sl], w1h_r[:, sl])
            nc.sync.dma_start(w1l_s[:, sl], w1l_r[:, sl])
            fb0 += nfb
            if si == 0:
                nc.sync.dma_start(b1_s[:], b1d[:])
            elif si == 1 and NC > 1:
                load_x(1)
        nc.sync.dma_start(wdv_s[:], wdv[:])

        b2w_s = const.tile([P, H], dt.float32)
        w2h_s = const.tile([P, HB, G, 2, 256], dt.float8e4)
        w2l_s = const.tile([P, HB, G, 2, 256], dt.float8e4)
        for hb in range(HB):
            nc.sync.dma_start(w2h_s[:, hb], w2h_r[:, hb])
            nc.sync.dma_start(w2l_s[:, hb], w2l_r[:, hb])
            if hb == 0 and not fuse2:
                nc.sync.dma_start(b2w_s[:], b2w[:])
            c = hb + 2
            if c < NC:
                load_x(c)
        for c in range(HB + 2, NC):
            load_x(c)

        hs = [None] * NC

        def alloc_h(c):
            h8 = h8pool.tile([P, G, 2, CK], dt.float8e4, name="h8")
            hl = hlpool.tile([P, G, 2, CK], dt.float8e4, name="hl")
            hs[c] = (h8, hl)

        def emit_mm1_group(c, fbp):
            csz = sizes[c]
            xht, xlt = xh_s[c], xl_s[c]
            h8, hl = hs[c]
            if True:
                ps = p1pool.tile([P, 2, CK], dt.float32, name="p1")
                for half in range(2):
                    fb = 2 * fbp + half
                    reg = ps[:, half, :csz]
                    for j in range(J):
                        nc.tensor.matmul(
                            reg, w1h_s[:, fb, j], xht[:, j, :, :csz],
                            start=(j == 0), stop=False, perf_mode=PM.DoubleRow)
                    for j in range(J):
                        nc.tensor.matmul(
                            reg, w1h_s[:, fb, j], xlt[:, j, :, :csz],
                            start=False, stop=False, perf_mode=PM.DoubleRow)
                    for j in range(J):
                        nc.tensor.matmul(
                            reg, w1l_s[:, fb, j], xht[:, j, :, :csz],
                            start=False, stop=(j == J - 1), perf_mode=PM.DoubleRow)
                g32 = gpool.tile([P, 2, CK], dt.float32, name="g32")
                h8v = h8[:, fbp, :, :csz]
                if fuse1:
                    nc.scalar.activation(
                        g32[:, :, :csz], ps[:, :, :csz], AF.Gelu,
                        bias=0.0, scale=1.0 / 256)
                    nc.scalar.activation(
                        h8v, ps[:, :, :csz], AF.Gelu, bias=0.0, scale=1.0 / 256)
                else:
                    for half in range(2):
                        fb = 2 * fbp + half
                        nc.scalar.activation(
                            g32[:, half, :csz], ps[:, half, :csz], AF.Gelu,
                            bias=b1_s[:, fb:fb + 1], scale=1.0 / 256)
                        nc.scalar.activation(
                            h8[:, fbp, half, :csz], ps[:, half, :csz], AF.Gelu,
                            bias=b1_s[:, fb:fb + 1], scale=1.0 / 256)
                nc.vector.tensor_tensor(
                    hl[:, fbp, :, :csz], g32[:, :, :csz], h8v, ALU.subtract)

        def emit_mm2(c, pair=True):
            csz = sizes[c]
            h8, hl = hs[c]
            for tt in range(csz // P):
                gt = c * 2 + tt
                t0 = tt * P
                for hbp in range(HB // (2 if pair else 1)):
                    nh = 2 if pair else 1
                    ps2 = p2pool.tile([P, 2, 256], dt.float32, name="p2")
                    for half in range(nh):
                        hb = nh * hbp + half
                        reg = ps2[:, half]
                        for g in range(G):
                            nc.tensor.matmul(
                                reg, h8[:, g, :, t0:t0 + P], w2h_s[:, hb, g],
                                start=(g == 0), stop=False, perf_mode=PM.DoubleRow)
                        for g in range(G):
                            nc.tensor.matmul(
                                reg, hl[:, g, :, t0:t0 + P], w2h_s[:, hb, g],
                                start=False, stop=False, perf_mode=PM.DoubleRow)
                        for g in range(G):
                            nc.tensor.matmul(
                                reg, h8[:, g, :, t0:t0 + P], w2l_s[:, hb, g],
                                start=False, stop=(g == G - 1), perf_mode=PM.DoubleRow)
                    wid = nh * 256
                    h0 = hbp * wid
                    ob = opool.tile([P, 2, 256], dt.float32, name="ob")
                    if fuse2:
                        nc.vector.tensor_scalar_mul(
                            ob[:, :nh], ps2[:, :nh], wdv_s[:, gt:gt + 1])
                    else:
                        nc.vector.tensor_tensor(
                            ob[:, :nh], ps2[:, :nh],
                            b2w_s[:, h0:h0 + wid].rearrange(
                                "p (n x) -> p n x", n=nh), ALU.add)
                        nc.vector.tensor_scalar_mul(
                            ob[:, :nh], ob[:, :nh], wdv_s[:, gt:gt + 1])
                    nc.sync.dma_start(
                        yc[gt * P:(gt + 1) * P, h0:h0 + wid],
                        ob[:, :nh].rearrange("p n x -> p (n x)"))

        def emit_mm1(c):
            alloc_h(c)
            for fbp in range(FB // 2):
                emit_mm1_group(c, fbp)

        # Software pipeline: the two head chunks' mm1s interleave by
        # fb-pair so each arriving w1 slice feeds two PE groups (PE
        # covers the w1 DMA stream with no idle); afterwards mm1 stays
        # two chunks ahead of mm2 so the w2 stream lands in time.
        if NC > 1:
            alloc_h(0)
            alloc_h(1)
            for fbp in range(FB // 2):
                emit_mm1_group(0, fbp)
                emit_mm1_group(1, fbp)
        else:
            emit_mm1(0)
        for c in range(NC):
            emit_mm2(c, pair=(c < NC - 1))
            if c + 2 < NC:
                emit_mm1(c + 2)
    return nc


def _get_nc(C, fuse1=True, fuse2=True):
    key = (C, fuse1, fuse2)
    if key not in _CACHE:
        nc = _build_nc(C, fuse1, fuse2)
        nc.finalize()
        _CACHE[key] = nc
    return _CACHE[key]


def _split8(a):
    hi = a.astype(fp8)
    lo = (a - hi.astype(np.float32)).astype(fp8)
    return hi, lo


def _x_layout(x8, idx, C):
    """[H, T] fp8 + token list -> [P, Cx*8] with [p, c, j, i, t] layout."""
    NC = (C + CK - 1) // CK
    Cx = NC * CK
    pad = np.zeros(Cx, dtype=np.int64)
    pad[:len(idx)] = idx
    g = x8[:, pad]                                   # [H, Cx]
    g = g.reshape(J, 2, P, NC, CK)                   # [j, i, p, c, t]
    return np.ascontiguousarray(
        g.transpose(2, 3, 0, 1, 4).reshape(P, Cx * 8))


def dispatch(hidden_states, router_w, router_b):
    """Host router: exact fp32 softmax top-2 + renormalized weights."""
    x = np.asarray(hidden_states, dtype=np.float32).reshape(T, H)
    logits = x @ np.asarray(router_w, dtype=np.float32)
    logits = logits + np.asarray(router_b, dtype=np.float32)
    part = np.argpartition(logits, E - 2, axis=1)[:, E - 2:]     # top-2 ids
    lg = np.take_along_axis(logits, part, axis=1)                # [T, 2]
    m = lg.max(axis=1, keepdims=True)
    e = np.exp(lg - m)
    wslot = e / e.sum(axis=1, keepdims=True)                     # [T, 2]
    idx_lists, wts = [], []
    for m_ in range(E):
        hit = part == m_
        rows = np.where(hit.any(axis=1))[0]
        idx_lists.append(rows)
        wts.append((wslot * hit)[rows].sum(axis=1))
    cmax = max(len(ix) for ix in idx_lists)
    C = max(P, ((cmax + P - 1) // P) * P)
    return x, idx_lists, wts, C


def make_in_maps(hidden_states, router_w, router_b, w1, b1, w2, b2):
    x, idx_lists, wts, C = dispatch(hidden_states, router_w, router_b)
    TTS = C // P
    xt = np.ascontiguousarray(x.T)                   # [H, T] f32
    x8h, x8l = _split8(xt)
    w1 = np.asarray(w1, dtype=np.float32)
    w2 = np.asarray(w2, dtype=np.float32)
    b1 = np.asarray(b1, dtype=np.float32)
    b2 = np.asarray(b2, dtype=np.float32)
    fuse1 = not b1.any()
    fuse2 = not b2.any()
    in_maps = []
    for m in range(E):
        ix = idx_lists[m]
        w1h, w1l = _split8(w1[m] * 256.0)            # [H, F]
        w2h, w2l = _split8(w2[m] * 256.0)            # [F, H]
        # [p, fb, j, i, f] = w1s[j*256+i*128+p, fb*128+f]
        w1m = [np.ascontiguousarray(
            a.reshape(J, 2, P, FB, P).transpose(2, 3, 0, 1, 4).reshape(P, -1))
            for a in (w1h, w1l)]
        # [p, hb, g, i, h] = w2s[(2g+i)*128+p, hb*256+h]
        w2m = [np.ascontiguousarray(
            a.reshape(G, 2, P, HB, 256).transpose(2, 3, 0, 1, 4).reshape(P, -1))
            for a in (w2h, w2l)]
        wcol = np.zeros(C, dtype=np.float32)
        wcol[:len(ix)] = wts[m] / 256.0
        in_maps.append({
            "xh": _x_layout(x8h, ix, C),
            "xl": _x_layout(x8l, ix, C),
            "w1h": w1m[0], "w1l": w1m[1],
            "w2h": w2m[0], "w2l": w2m[1],
            "b1d": np.ascontiguousarray(b1[m].reshape(FB, P).T),
            "b2w": np.ascontiguousarray(
                np.broadcast_to(b2[m] * 256.0, (P, H)).astype(np.float32)),
            "wdv": np.ascontiguousarray(wcol.reshape(TTS, P).T),
        })
    return in_maps, idx_lists, C, fuse1, fuse2


def run_device(in_maps, C, fuse1=True, fuse2=True):
    from concourse.bass_utils import run_bass_kernel_spmd

    nc = _get_nc(C, fuse1, fuse2)
    res = run_bass_kernel_spmd(nc, in_maps, core_ids=list(range(E)))
    return res.results


def kernel(hidden_states, router_w, router_b, w1, b1, w2, b2):
    in_maps, idx_lists, C, fuse1, fuse2 = make_in_maps(
        hidden_states, router_w, router_b, w1, b1, w2, b2)
    # One retry guards against a rare transient execution glitch observed on
    # the very first load of a freshly compiled NEFF (garbage ~1e35 values);
    # a healthy output has absmax of a few units.
    last_err = None
    acc = None
    for attempt in range(3):
        try:
            results = run_device(in_maps, C, fuse1, fuse2)
        except Exception as e:  # transient NRT/axon failures observed
            last_err = e
            import time as _time
            _time.sleep(10)
            continue
        acc = np.zeros((T, H), dtype=np.float32)
        for m in range(E):
            ix = idx_lists[m]
            acc[ix] += np.asarray(results[m]["yc"], dtype=np.float32)[:len(ix)]
        if np.isfinite(acc).all() and np.abs(acc).max() < 1e4:
            return acc.reshape(B, S, H)
    if acc is None and last_err is not None:
        raise last_err
    return acc.reshape(B, S, H)
